# revision 12
# baseline (speedup 1.0000x reference)
"""Trainium2 Bass kernel for nn_Detector_head (SuperPoint-style detector head).

Pipeline per sample: 3x3 conv(256->256)+BN+ReLU -> 1x1 conv(256->65)+BN ->
softmax(65) -> drop dustbin -> pixel_shuffle(8) -> greedy box-NMS -> top-300.

Sharding: pure data parallelism, batch 32 -> 8 cores x 4 samples.
"""

import sys

sys.path.insert(0, "/opt/trn_rl_repo")

import numpy as np

B_PER_CORE = 4
CIN = 256
CMID = 256
COUT = 65
H, W = 60, 80
PIX = H * W  # 4800
HP, WP = H + 2, W + 2  # 62, 82
PPIX = HP * WP  # 5084
GRID = 8
HH, HW_ = H * GRID, W * GRID  # 480, 640
HEAT_N = HH * HW_  # 307200
NTILE = 10  # pixel tiles for conv (480 each)
TILE_PIX = PIX // NTILE  # 480
TROWS = TILE_PIX // W  # 6 rows per tile
EPS = 1e-5

NMS_SIZE = 4.0
IOU_TH = 0.1
MIN_PROB = 0.015
TOP_K = 300
NMS_CAND = 1024

_CACHED = {}


def _last_in_maps_get():
    return _CACHED.get("last_in_maps")


def _build_nc():
    import concourse.bacc as bacc
    import concourse.mybir as mybir
    from concourse.tile import TileContext

    f32 = mybir.dt.float32
    bf16 = mybir.dt.bfloat16
    nc = bacc.Bacc("TRN2", target_bir_lowering=False, debug=False, num_devices=8)

    xh_ext = nc.declare_dram_parameter(
        "x_hi", [B_PER_CORE, CIN, H, W], bf16, isOutput=False
    )
    xl_ext = nc.declare_dram_parameter(
        "x_lo", [B_PER_CORE, CIN, H, W], bf16, isOutput=False
    )
    wa_ext = nc.declare_dram_parameter("Wa", [CMID, CIN * 9], f32, isOutput=False)
    wb_ext = nc.declare_dram_parameter("Wb", [COUT, CMID], f32, isOutput=False)
    sa_ext = nc.declare_dram_parameter("scale_a", [CMID], f32, isOutput=False)
    ba_ext = nc.declare_dram_parameter("bias_a", [CMID], f32, isOutput=False)
    sb_ext = nc.declare_dram_parameter("scale_b", [COUT, 1], f32, isOutput=False)
    bb_ext = nc.declare_dram_parameter("bias_b", [COUT, 1], f32, isOutput=False)
    ones_ext = nc.declare_dram_parameter("ones128", [128, 128], f32, isOutput=False)
    ident_ext = nc.declare_dram_parameter("ident128", [128, 128], f32, isOutput=False)

    s_dram = nc.dram_tensor("s_scratch", [PIX], f32)
    logits_out = nc.declare_dram_parameter(
        "logits", [B_PER_CORE, COUT, H, W], f32, isOutput=True
    )
    heat_out = nc.declare_dram_parameter(
        "heat", [B_PER_CORE, HH, HW_], f32, isOutput=True
    )

    with TileContext(nc) as tc:
        with (
            tc.tile_pool(name="const", bufs=1) as cpool,
            tc.tile_pool(name="wts", bufs=1) as wpool,
            tc.tile_pool(name="xp", bufs=1) as xpool,
            tc.tile_pool(name="hb", bufs=1) as hpool,
            tc.tile_pool(name="cm", bufs=1) as cmpool,
            tc.tile_pool(name="wk", bufs=2) as wkpool,
            tc.tile_pool(name="ps", bufs=3, space="PSUM") as pspool,
            tc.tile_pool(name="ps1", bufs=2, space="PSUM") as ps1pool,
        ):
            ones_sb = cpool.tile([128, 128], f32)
            ident_sb = cpool.tile([128, 128], f32)
            nc.sync.dma_start(out=ones_sb[:], in_=ones_ext[:])
            nc.sync.dma_start(out=ident_sb[:], in_=ident_ext[:])

            # ---- weight prep ----
            # Wa laid as [co(2 chunks of 128 partitions), ci*9]; scale folded in.
            wa_sb = wpool.tile([128, 2, CIN * 9], f32)
            nc.sync.dma_start(
                out=wa_sb[:],
                in_=wa_ext.ap().rearrange("(m p) k -> p m k", p=128),
            )
            sa_sb = wpool.tile([128, 2], f32)
            nc.sync.dma_start(
                out=sa_sb[:], in_=sa_ext.ap().rearrange("(m p) -> p m", p=128)
            )
            ba_sb = wpool.tile([128, 2], f32)
            nc.sync.dma_start(
                out=ba_sb[:], in_=ba_ext.ap().rearrange("(m p) -> p m", p=128)
            )
            for m in range(2):
                nc.vector.tensor_scalar_mul(
                    wa_sb[:, m, :], wa_sb[:, m, :], sa_sb[:, m : m + 1]
                )
            # transpose -> waT split into bf16 hi/lo pair (3-pass fp32 matmul)
            waT_hi = wpool.tile([128, 18, CMID], bf16)
            waT_lo = wpool.tile([128, 18, CMID], bf16)
            for m in range(2):
                for k in range(2):
                    for t in range(9):
                        pst = pspool.tile([128, 128], f32, tag="mm")
                        srcap = wa_sb[:, m, :].rearrange(
                            "p (c nine) -> p c nine", nine=9
                        )
                        nc.tensor.transpose(
                            pst[:], srcap[:, k * 128 : (k + 1) * 128, t], ident_sb[:]
                        )
                        dst_hi = waT_hi[:, k * 9 + t, m * 128 : (m + 1) * 128]
                        dst_lo = waT_lo[:, k * 9 + t, m * 128 : (m + 1) * 128]
                        scr = wkpool.tile([128, 128], f32, tag="wscr")
                        nc.vector.tensor_copy(dst_hi, pst[:])  # f32 -> bf16 round
                        nc.vector.tensor_copy(scr[:], dst_hi)  # bf16 -> f32 exact
                        nc.vector.tensor_sub(scr[:], pst[:], scr[:])
                        nc.vector.tensor_copy(dst_lo, scr[:])  # residual -> bf16

            # Wb [65, 256] scaled then transposed -> wbT [ci 128, k 2, 65]
            wb_sb = wpool.tile([COUT, CMID], f32)
            nc.sync.dma_start(out=wb_sb[:], in_=wb_ext[:])
            sb_sb = wpool.tile([COUT, 1], f32)
            nc.sync.dma_start(out=sb_sb[:], in_=sb_ext[:])
            bb_sb = wpool.tile([COUT, 1], f32)
            nc.sync.dma_start(out=bb_sb[:], in_=bb_ext[:])
            nc.vector.tensor_scalar_mul(wb_sb[:], wb_sb[:], sb_sb[:])
            wbT = wpool.tile([128, 2, COUT], f32)
            for k in range(2):
                pst = pspool.tile([128, COUT], f32, tag="mm")
                nc.tensor.transpose(
                    pst[:], wb_sb[:, k * 128 : (k + 1) * 128], ident_sb[:COUT, :COUT]
                )
                nc.vector.tensor_copy(wbT[:, k, :], pst[:])

            # padded input tiles (border zeroed once; interior rewritten per sample)
            x_ph = xpool.tile([128, 2, PPIX], bf16)
            x_pl = xpool.tile([128, 2, PPIX], bf16)
            nc.vector.memset(x_ph[:], 0.0)
            nc.vector.memset(x_pl[:], 0.0)

            h_sb = hpool.tile([128, 2, PIX], f32)
            logits_cm = cmpool.tile([COUT, PIX], f32)
            work_cm = cmpool.tile([COUT, PIX], f32)
            rb_cm = cmpool.tile([COUT, PIX], f32)
            s_row = cmpool.tile([1, PIX], f32)

            for b in range(B_PER_CORE):
                # load x hi/lo into padded interiors (one DMA per ci chunk)
                for xt, xe in ((x_ph, xh_ext), (x_pl, xl_ext)):
                    for k in range(2):
                        nc.sync.dma_start(
                            out=xt[:, k, :].rearrange("p (hh ww) -> p hh ww", hh=HP)[
                                :, 1 : 1 + H, 1 : 1 + W
                            ],
                            in_=xe.ap()[b].rearrange(
                                "(k p) hh ww -> k p hh ww", p=128
                            )[k],
                        )
                # ---- conv-a (3x3) + BN + ReLU ----
                for m in range(2):
                    for tl in range(NTILE):
                        pa = pspool.tile([128, TILE_PIX], f32, tag="mm")
                        y0 = tl * TROWS
                        n_mm = 0
                        for k in range(2):
                            for t in range(9):
                                dy, dx = t // 3, t % 3

                                def _rhs(xt):
                                    return xt[:, k, :].rearrange(
                                        "p (hh ww) -> p hh ww", hh=HP
                                    )[:, y0 + dy : y0 + dy + TROWS, dx : dx + W]

                                wslice_hi = waT_hi[
                                    :, k * 9 + t, m * 128 : (m + 1) * 128
                                ]
                                wslice_lo = waT_lo[
                                    :, k * 9 + t, m * 128 : (m + 1) * 128
                                ]
                                for lhs, rhs in (
                                    (wslice_hi, _rhs(x_ph)),
                                    (wslice_hi, _rhs(x_pl)),
                                    (wslice_lo, _rhs(x_ph)),
                                ):
                                    nc.tensor.matmul(
                                        pa[:],
                                        lhs,
                                        rhs,
                                        start=(n_mm == 0),
                                        stop=(n_mm == 53),
                                    )
                                    n_mm += 1
                        nc.scalar.activation(
                            h_sb[:, m, tl * TILE_PIX : (tl + 1) * TILE_PIX],
                            pa[:],
                            mybir.ActivationFunctionType.Relu,
                            bias=ba_sb[:, m : m + 1],
                            scale=1.0,
                        )

                # ---- conv-b (1x1) + BN  (channel-major) ----
                for tl in range(NTILE):
                    pb = pspool.tile([COUT, TILE_PIX], f32, tag="mm")
                    for k in range(2):
                        nc.tensor.matmul(
                            pb[:],
                            wbT[:, k, :],
                            h_sb[:, k, tl * TILE_PIX : (tl + 1) * TILE_PIX],
                            start=(k == 0),
                            stop=(k == 1),
                        )
                    nc.vector.tensor_scalar_add(
                        logits_cm[:, tl * TILE_PIX : (tl + 1) * TILE_PIX],
                        pb[:],
                        bb_sb[:],
                    )
                nc.sync.dma_start(
                    out=logits_out.ap()[b].rearrange("c hh ww -> c (hh ww)"),
                    in_=logits_cm[:],
                )

                # ---- softmax over channel dim ----
                nc.scalar.activation(
                    work_cm[:], logits_cm[:], mybir.ActivationFunctionType.Exp
                )
                for tl in range(NTILE):
                    psum_s = ps1pool.tile([1, TILE_PIX], f32, tag="sm")
                    nc.tensor.matmul(
                        psum_s[:],
                        ones_sb[:COUT, 0:1],
                        work_cm[:, tl * TILE_PIX : (tl + 1) * TILE_PIX],
                        start=True,
                        stop=True,
                    )
                    nc.vector.tensor_copy(
                        s_row[:, tl * TILE_PIX : (tl + 1) * TILE_PIX], psum_s[:]
                    )
                nc.vector.reciprocal(s_row[:], s_row[:])
                # broadcast 1/S to all channel partitions: bounce via DRAM with a
                # 0-stride read on the DRAM side
                nc.sync.dma_start(out=s_dram.ap()[None, :], in_=s_row[:])
                nc.sync.dma_start(
                    out=rb_cm[:], in_=s_dram.ap()[None, :].broadcast_to([COUT, PIX])
                )
                nc.vector.tensor_mul(work_cm[:], work_cm[:], rb_cm[:])

                # ---- heat output: pixel shuffle via DMA access pattern ----
                # heat[b, hc*8+r, wc*8+cc] = prob[r*8+cc, hc*80+wc]
                dma_engines = (nc.sync, nc.scalar, nc.gpsimd)
                for r in range(GRID):
                    for cc in range(GRID):
                        c = r * GRID + cc
                        dma_engines[c % len(dma_engines)].dma_start(
                            out=heat_out.ap()[b].rearrange(
                                "(hc r) (wc cc) -> r cc hc wc", r=GRID, cc=GRID
                            )[r, cc],
                            in_=work_cm[c : c + 1, :].rearrange(
                                "c (hc wc) -> c hc wc", hc=H
                            ),
                        )

    nc.finalize()
    return nc


def _get_nc():
    if "nc" not in _CACHED:
        _CACHED["nc"] = _build_nc()
    return _CACHED["nc"]


def _host_nms(heat):
    """Exact replication of reference _box_nms (vectorized, Jacobi to fixpoint)."""
    B = heat.shape[0]
    flat = heat.reshape(B, -1)
    # top-1024 sorted desc, ties by index asc (matches jax top_k)
    idx = np.argsort(-flat, axis=1, kind="stable")[:, :NMS_CAND]
    scores = np.take_along_axis(flat, idx, axis=1)
    ys = (idx // HW_).astype(np.float32)
    xs = (idx % HW_).astype(np.float32)
    heat_nms = np.zeros_like(flat)
    for b in range(B):
        dy = np.abs(ys[b][:, None] - ys[b][None, :])
        dx = np.abs(xs[b][:, None] - xs[b][None, :])
        inter = np.maximum(NMS_SIZE - dy, 0.0) * np.maximum(NMS_SIZE - dx, 0.0)
        iou = inter / (2.0 * NMS_SIZE * NMS_SIZE - inter)
        overlap = iou > IOU_TH
        valid = scores[b] > MIN_PROB
        np.fill_diagonal(overlap, False)
        ov_ut = np.triu(overlap, 1)
        keep = valid.copy()
        for _ in range(NMS_CAND + 1):  # Jacobi fixpoint == greedy result;
            supp = ov_ut[keep].any(axis=0)  # converges in <= chain depth iters
            newkeep = valid & ~supp
            if (newkeep == keep).all():
                break
            keep = newkeep
        rank = np.cumsum(keep)
        keep = keep & (rank <= TOP_K)
        kept = np.where(keep, scores[b], 0.0).astype(np.float32)
        heat_nms[b, idx[b]] = kept
    return heat_nms.reshape(B, HH, HW_)


def kernel(x, Wa, ba, ga, bta, ma, va, Wb, bb, gb, btb, mb, vb):
    from concourse.bass_utils import run_bass_kernel_spmd

    import ml_dtypes

    x = np.ascontiguousarray(np.asarray(x, dtype=np.float32))
    x_hi = x.astype(ml_dtypes.bfloat16)
    x_lo = (x - x_hi.astype(np.float32)).astype(ml_dtypes.bfloat16)
    nc = _get_nc()

    # fold BN params on host (cheap per-channel math, not data-dependent)
    def bn_fold(g, v, m_, bt, bconv):
        g = np.asarray(g, np.float32)
        v = np.asarray(v, np.float32)
        m_ = np.asarray(m_, np.float32)
        bt = np.asarray(bt, np.float32)
        bconv = np.asarray(bconv, np.float32)
        scale = (g * (1.0 / np.sqrt(v + np.float32(EPS)))).astype(np.float32)
        bias = ((bconv - m_) * scale + bt).astype(np.float32)
        return scale, bias

    scale_a, bias_a = bn_fold(ga, va, ma, bta, ba)
    scale_b, bias_b = bn_fold(gb, vb, mb, btb, bb)

    WaR = np.ascontiguousarray(np.asarray(Wa, np.float32).reshape(CMID, CIN, 9)
                               .transpose(0, 1, 2).reshape(CMID, CIN * 9))
    WbR = np.ascontiguousarray(np.asarray(Wb, np.float32).reshape(COUT, CMID))

    ones128 = np.ones((128, 128), np.float32)
    ident128 = np.eye(128, dtype=np.float32)

    in_maps = []
    for c in range(8):
        in_maps.append(
            {
                "x_hi": x_hi[c * B_PER_CORE : (c + 1) * B_PER_CORE],
                "x_lo": x_lo[c * B_PER_CORE : (c + 1) * B_PER_CORE],
                "Wa": WaR,
                "Wb": WbR,
                "scale_a": scale_a,
                "bias_a": bias_a,
                "scale_b": scale_b.reshape(COUT, 1),
                "bias_b": bias_b.reshape(COUT, 1),
                "ones128": ones128,
                "ident128": ident128,
            }
        )

    _CACHED["last_in_maps"] = in_maps
    res = run_bass_kernel_spmd(nc, in_maps, core_ids=list(range(8)))
    logits = np.concatenate([r["logits"] for r in res.results], axis=0)
    heat = np.concatenate([r["heat"] for r in res.results], axis=0)

    heat_nms = _host_nms(heat)
    pred = (heat_nms >= MIN_PROB).astype(np.int32)
    return logits, heat, heat_nms, pred


# revision 15
# speedup vs baseline: 1.3181x; 1.3181x over previous
"""Trainium2 Bass kernel for nn_Detector_head (SuperPoint-style detector head).

Pipeline per sample: 3x3 conv(256->256)+BN+ReLU -> 1x1 conv(256->65)+BN ->
softmax(65) -> drop dustbin -> pixel_shuffle(8) -> greedy box-NMS -> top-300.

Sharding: pure data parallelism, batch 32 -> 8 cores x 4 samples.
"""

import sys

sys.path.insert(0, "/opt/trn_rl_repo")

import numpy as np

B_PER_CORE = 4
CIN = 256
CMID = 256
COUT = 65
H, W = 60, 80
PIX = H * W  # 4800
HP, WP = H + 2, W + 2  # 62, 82
PPIX = HP * WP  # 5084
GRID = 8
HH, HW_ = H * GRID, W * GRID  # 480, 640
HEAT_N = HH * HW_  # 307200
NTILE = 10  # pixel tiles for conv (480 each)
TILE_PIX = PIX // NTILE  # 480
TROWS = TILE_PIX // W  # 6 rows per tile
EPS = 1e-5

NMS_SIZE = 4.0
IOU_TH = 0.1
MIN_PROB = 0.015
TOP_K = 300
NMS_CAND = 1024

_CACHED = {}


def _last_in_maps_get():
    return _CACHED.get("last_in_maps")


def _build_nc():
    import concourse.bacc as bacc
    import concourse.mybir as mybir
    from concourse.tile import TileContext

    f32 = mybir.dt.float32
    bf16 = mybir.dt.bfloat16
    nc = bacc.Bacc("TRN2", target_bir_lowering=False, debug=False, num_devices=8)

    xh_ext = nc.declare_dram_parameter(
        "x_hi", [B_PER_CORE, CIN, H, W], bf16, isOutput=False
    )
    xl_ext = nc.declare_dram_parameter(
        "x_lo", [B_PER_CORE, CIN, H, W], bf16, isOutput=False
    )
    wa_ext = nc.declare_dram_parameter("Wa", [CMID, CIN * 9], f32, isOutput=False)
    wb_ext = nc.declare_dram_parameter("Wb", [COUT, CMID], f32, isOutput=False)
    sa_ext = nc.declare_dram_parameter("scale_a", [CMID], f32, isOutput=False)
    ba_ext = nc.declare_dram_parameter("bias_a", [CMID], f32, isOutput=False)
    sb_ext = nc.declare_dram_parameter("scale_b", [COUT, 1], f32, isOutput=False)
    bb_ext = nc.declare_dram_parameter("bias_b", [COUT, 1], f32, isOutput=False)
    ones_ext = nc.declare_dram_parameter("ones128", [128, 128], f32, isOutput=False)
    ident_ext = nc.declare_dram_parameter("ident128", [128, 128], f32, isOutput=False)

    s_dram = nc.dram_tensor("s_scratch", [B_PER_CORE, PIX], f32)
    logits_out = nc.declare_dram_parameter(
        "logits", [B_PER_CORE, COUT, H, W], f32, isOutput=True
    )
    heat_out = nc.declare_dram_parameter(
        "heat", [B_PER_CORE, HH, HW_], f32, isOutput=True
    )

    with TileContext(nc) as tc:
        with (
            tc.tile_pool(name="const", bufs=1) as cpool,
            tc.tile_pool(name="wts", bufs=1) as wpool,
            tc.tile_pool(name="xp", bufs=1) as xpool,
            tc.tile_pool(name="hb", bufs=1) as hpool,
            tc.tile_pool(name="cm", bufs=1) as cmpool,
            tc.tile_pool(name="wk", bufs=2) as wkpool,
            tc.tile_pool(name="ps", bufs=5, space="PSUM") as pspool,
            tc.tile_pool(name="ps1", bufs=2, space="PSUM") as ps1pool,
        ):
            ones_sb = cpool.tile([128, 128], f32)
            ident_sb = cpool.tile([128, 128], f32)
            nc.sync.dma_start(out=ones_sb[:], in_=ones_ext[:])
            nc.sync.dma_start(out=ident_sb[:], in_=ident_ext[:])

            # ---- weight prep ----
            # Wa laid as [co(2 chunks of 128 partitions), ci*9]; scale folded in.
            wa_sb = wpool.tile([128, 2, CIN * 9], f32)
            nc.sync.dma_start(
                out=wa_sb[:],
                in_=wa_ext.ap().rearrange("(m p) k -> p m k", p=128),
            )
            sa_sb = wpool.tile([128, 2], f32)
            nc.sync.dma_start(
                out=sa_sb[:], in_=sa_ext.ap().rearrange("(m p) -> p m", p=128)
            )
            ba_sb = wpool.tile([128, 2], f32)
            nc.sync.dma_start(
                out=ba_sb[:], in_=ba_ext.ap().rearrange("(m p) -> p m", p=128)
            )
            for m in range(2):
                nc.vector.tensor_scalar_mul(
                    wa_sb[:, m, :], wa_sb[:, m, :], sa_sb[:, m : m + 1]
                )
            # transpose -> waT split into bf16 hi/lo pair (3-pass fp32 matmul)
            waT_hi = wpool.tile([128, 18, CMID], bf16)
            waT_lo = wpool.tile([128, 18, CMID], bf16)
            for m in range(2):
                for k in range(2):
                    for t in range(9):
                        pst = pspool.tile([128, 128], f32, tag="mm")
                        srcap = wa_sb[:, m, :].rearrange(
                            "p (c nine) -> p c nine", nine=9
                        )
                        nc.tensor.transpose(
                            pst[:], srcap[:, k * 128 : (k + 1) * 128, t], ident_sb[:]
                        )
                        dst_hi = waT_hi[:, k * 9 + t, m * 128 : (m + 1) * 128]
                        dst_lo = waT_lo[:, k * 9 + t, m * 128 : (m + 1) * 128]
                        scr = wkpool.tile([128, 128], f32, tag="wscr")
                        nc.scalar.copy(dst_hi, pst[:])  # f32 -> bf16 round
                        nc.scalar.copy(scr[:], dst_hi)  # bf16 -> f32 exact
                        nc.vector.tensor_sub(scr[:], pst[:], scr[:])
                        nc.vector.tensor_copy(dst_lo, scr[:])  # residual -> bf16

            # Wb [65, 256] scaled then transposed -> wbT [ci 128, k 2, 65]
            wb_sb = wpool.tile([COUT, CMID], f32)
            nc.sync.dma_start(out=wb_sb[:], in_=wb_ext[:])
            sb_sb = wpool.tile([COUT, 1], f32)
            nc.sync.dma_start(out=sb_sb[:], in_=sb_ext[:])
            bb_sb = wpool.tile([COUT, 1], f32)
            nc.sync.dma_start(out=bb_sb[:], in_=bb_ext[:])
            nc.vector.tensor_scalar_mul(wb_sb[:], wb_sb[:], sb_sb[:])
            wbT_hi = wpool.tile([128, 2, COUT], bf16)
            wbT_lo = wpool.tile([128, 2, COUT], bf16)
            for k in range(2):
                pst = pspool.tile([128, COUT], f32, tag="mm")
                nc.tensor.transpose(
                    pst[:], wb_sb[:, k * 128 : (k + 1) * 128], ident_sb[:COUT, :COUT]
                )
                scrb = wkpool.tile([128, COUT], f32, tag="wscr2")
                nc.scalar.copy(wbT_hi[:, k, :], pst[:])
                nc.scalar.copy(scrb[:], wbT_hi[:, k, :])
                nc.vector.tensor_sub(scrb[:], pst[:], scrb[:])
                nc.vector.tensor_copy(wbT_lo[:, k, :], scrb[:])

            # padded input tiles (border zeroed once; interior rewritten per sample)
            x_ph = xpool.tile([128, 2, PPIX], bf16)
            x_pl = xpool.tile([128, 2, PPIX], bf16)
            nc.vector.memset(x_ph[:], 0.0)
            nc.vector.memset(x_pl[:], 0.0)

            h_hi = hpool.tile([128, 2, PIX], bf16)
            h_lo = hpool.tile([128, 2, PIX], bf16)
            logits_cm = cmpool.tile([COUT, PIX], f32)
            s_row = cmpool.tile([1, PIX], f32)

            for b in range(B_PER_CORE):
                work_cm = wkpool.tile([COUT, PIX], f32, tag="work")
                # load x hi/lo into padded interiors (one DMA per ci chunk)
                for xt, xe in ((x_ph, xh_ext), (x_pl, xl_ext)):
                    for k in range(2):
                        nc.sync.dma_start(
                            out=xt[:, k, :].rearrange("p (hh ww) -> p hh ww", hh=HP)[
                                :, 1 : 1 + H, 1 : 1 + W
                            ],
                            in_=xe.ap()[b].rearrange(
                                "(k p) hh ww -> k p hh ww", p=128
                            )[k],
                        )
                # ---- conv-a (3x3) + BN + ReLU ----
                for m in range(2):
                    for tl in range(NTILE):
                        pa = pspool.tile([128, TILE_PIX], f32, tag="mm")
                        y0 = tl * TROWS
                        n_mm = 0
                        for k in range(2):
                            for t in range(9):
                                dy, dx = t // 3, t % 3

                                def _rhs(xt):
                                    return xt[:, k, :].rearrange(
                                        "p (hh ww) -> p hh ww", hh=HP
                                    )[:, y0 + dy : y0 + dy + TROWS, dx : dx + W]

                                wslice_hi = waT_hi[
                                    :, k * 9 + t, m * 128 : (m + 1) * 128
                                ]
                                wslice_lo = waT_lo[
                                    :, k * 9 + t, m * 128 : (m + 1) * 128
                                ]
                                for lhs, rhs in (
                                    (wslice_hi, _rhs(x_ph)),
                                    (wslice_hi, _rhs(x_pl)),
                                    (wslice_lo, _rhs(x_ph)),
                                ):
                                    nc.tensor.matmul(
                                        pa[:],
                                        lhs,
                                        rhs,
                                        start=(n_mm == 0),
                                        stop=(n_mm == 53),
                                    )
                                    n_mm += 1
                        hs = wkpool.tile([128, TILE_PIX], f32, tag="hscr")
                        hs2 = wkpool.tile([128, TILE_PIX], f32, tag="hscr2")
                        sl = slice(tl * TILE_PIX, (tl + 1) * TILE_PIX)
                        nc.scalar.activation(
                            hs[:],
                            pa[:],
                            mybir.ActivationFunctionType.Relu,
                            bias=ba_sb[:, m : m + 1],
                            scale=1.0,
                        )
                        nc.scalar.copy(h_hi[:, m, sl], hs[:])
                        nc.vector.tensor_copy(hs2[:], h_hi[:, m, sl])
                        nc.vector.tensor_sub(hs2[:], hs[:], hs2[:])
                        nc.vector.tensor_copy(h_lo[:, m, sl], hs2[:])

                # ---- conv-b (1x1) + BN  (channel-major) ----
                for tl in range(NTILE):
                    pb = pspool.tile([COUT, TILE_PIX], f32, tag="mm")
                    n_mm = 0
                    for k in range(2):
                        hsl = slice(tl * TILE_PIX, (tl + 1) * TILE_PIX)
                        for lhs, rhs in (
                            (wbT_hi[:, k, :], h_hi[:, k, hsl]),
                            (wbT_hi[:, k, :], h_lo[:, k, hsl]),
                            (wbT_lo[:, k, :], h_hi[:, k, hsl]),
                        ):
                            nc.tensor.matmul(
                                pb[:], lhs, rhs, start=(n_mm == 0), stop=(n_mm == 5)
                            )
                            n_mm += 1
                    nc.vector.tensor_scalar_add(
                        logits_cm[:, tl * TILE_PIX : (tl + 1) * TILE_PIX],
                        pb[:],
                        bb_sb[:],
                    )
                nc.sync.dma_start(
                    out=logits_out.ap()[b].rearrange("c hh ww -> c (hh ww)"),
                    in_=logits_cm[:],
                )

                # ---- softmax over channel dim ----
                nc.scalar.activation(
                    work_cm[:], logits_cm[:], mybir.ActivationFunctionType.Exp
                )
                for tl in range(NTILE):
                    psum_s = ps1pool.tile([1, TILE_PIX], f32, tag="sm")
                    nc.tensor.matmul(
                        psum_s[:],
                        ones_sb[:COUT, 0:1],
                        work_cm[:, tl * TILE_PIX : (tl + 1) * TILE_PIX],
                        start=True,
                        stop=True,
                    )
                    nc.vector.tensor_copy(
                        s_row[:, tl * TILE_PIX : (tl + 1) * TILE_PIX], psum_s[:]
                    )
                nc.vector.reciprocal(s_row[:], s_row[:])
                # broadcast 1/S to all channel partitions: bounce via DRAM with a
                # 0-stride read on the DRAM side; logits_cm is dead after its
                # output DMA, so reuse it as the broadcast target
                nc.sync.dma_start(out=s_dram.ap()[b][None, :], in_=s_row[:])
                nc.sync.dma_start(
                    out=logits_cm[:],
                    in_=s_dram.ap()[b][None, :].broadcast_to([COUT, PIX]),
                )
                nc.vector.tensor_mul(work_cm[:], work_cm[:], logits_cm[:])

                # ---- heat output: pixel shuffle via DMA access pattern ----
                # heat[b, hc*8+r, wc*8+cc] = prob[r*8+cc, hc*80+wc]
                dma_engines = (nc.sync, nc.scalar, nc.gpsimd)
                for r in range(GRID):
                    for cc in range(GRID):
                        c = r * GRID + cc
                        dma_engines[c % len(dma_engines)].dma_start(
                            out=heat_out.ap()[b].rearrange(
                                "(hc r) (wc cc) -> r cc hc wc", r=GRID, cc=GRID
                            )[r, cc],
                            in_=work_cm[c : c + 1, :].rearrange(
                                "c (hc wc) -> c hc wc", hc=H
                            ),
                        )

    nc.finalize()
    return nc


def _get_nc():
    if "nc" not in _CACHED:
        _CACHED["nc"] = _build_nc()
    return _CACHED["nc"]


def _host_nms(heat):
    """Exact replication of reference _box_nms (vectorized, Jacobi to fixpoint)."""
    B = heat.shape[0]
    flat = heat.reshape(B, -1)
    # top-1024 sorted desc, ties by index asc (matches jax top_k)
    idx = np.argsort(-flat, axis=1, kind="stable")[:, :NMS_CAND]
    scores = np.take_along_axis(flat, idx, axis=1)
    ys = (idx // HW_).astype(np.float32)
    xs = (idx % HW_).astype(np.float32)
    heat_nms = np.zeros_like(flat)
    for b in range(B):
        dy = np.abs(ys[b][:, None] - ys[b][None, :])
        dx = np.abs(xs[b][:, None] - xs[b][None, :])
        inter = np.maximum(NMS_SIZE - dy, 0.0) * np.maximum(NMS_SIZE - dx, 0.0)
        iou = inter / (2.0 * NMS_SIZE * NMS_SIZE - inter)
        overlap = iou > IOU_TH
        valid = scores[b] > MIN_PROB
        np.fill_diagonal(overlap, False)
        ov_ut = np.triu(overlap, 1)
        keep = valid.copy()
        for _ in range(NMS_CAND + 1):  # Jacobi fixpoint == greedy result;
            supp = ov_ut[keep].any(axis=0)  # converges in <= chain depth iters
            newkeep = valid & ~supp
            if (newkeep == keep).all():
                break
            keep = newkeep
        rank = np.cumsum(keep)
        keep = keep & (rank <= TOP_K)
        kept = np.where(keep, scores[b], 0.0).astype(np.float32)
        heat_nms[b, idx[b]] = kept
    return heat_nms.reshape(B, HH, HW_)


def kernel(x, Wa, ba, ga, bta, ma, va, Wb, bb, gb, btb, mb, vb):
    from concourse.bass_utils import run_bass_kernel_spmd

    import ml_dtypes

    x = np.ascontiguousarray(np.asarray(x, dtype=np.float32))
    x_hi = x.astype(ml_dtypes.bfloat16)
    x_lo = (x - x_hi.astype(np.float32)).astype(ml_dtypes.bfloat16)
    nc = _get_nc()

    # fold BN params on host (cheap per-channel math, not data-dependent)
    def bn_fold(g, v, m_, bt, bconv):
        g = np.asarray(g, np.float32)
        v = np.asarray(v, np.float32)
        m_ = np.asarray(m_, np.float32)
        bt = np.asarray(bt, np.float32)
        bconv = np.asarray(bconv, np.float32)
        scale = (g * (1.0 / np.sqrt(v + np.float32(EPS)))).astype(np.float32)
        bias = ((bconv - m_) * scale + bt).astype(np.float32)
        return scale, bias

    scale_a, bias_a = bn_fold(ga, va, ma, bta, ba)
    scale_b, bias_b = bn_fold(gb, vb, mb, btb, bb)

    WaR = np.ascontiguousarray(np.asarray(Wa, np.float32).reshape(CMID, CIN, 9)
                               .transpose(0, 1, 2).reshape(CMID, CIN * 9))
    WbR = np.ascontiguousarray(np.asarray(Wb, np.float32).reshape(COUT, CMID))

    ones128 = np.ones((128, 128), np.float32)
    ident128 = np.eye(128, dtype=np.float32)

    in_maps = []
    for c in range(8):
        in_maps.append(
            {
                "x_hi": x_hi[c * B_PER_CORE : (c + 1) * B_PER_CORE],
                "x_lo": x_lo[c * B_PER_CORE : (c + 1) * B_PER_CORE],
                "Wa": WaR,
                "Wb": WbR,
                "scale_a": scale_a,
                "bias_a": bias_a,
                "scale_b": scale_b.reshape(COUT, 1),
                "bias_b": bias_b.reshape(COUT, 1),
                "ones128": ones128,
                "ident128": ident128,
            }
        )

    _CACHED["last_in_maps"] = in_maps
    res = run_bass_kernel_spmd(nc, in_maps, core_ids=list(range(8)))
    logits = np.concatenate([r["logits"] for r in res.results], axis=0)
    heat = np.concatenate([r["heat"] for r in res.results], axis=0)

    heat_nms = _host_nms(heat)
    pred = (heat_nms >= MIN_PROB).astype(np.int32)
    return logits, heat, heat_nms, pred


# revision 17
# speedup vs baseline: 1.3582x; 1.0304x over previous
"""Trainium2 Bass kernel for nn_Detector_head (SuperPoint-style detector head).

Pipeline per sample: 3x3 conv(256->256)+BN+ReLU -> 1x1 conv(256->65)+BN ->
softmax(65) -> drop dustbin -> pixel_shuffle(8) -> greedy box-NMS -> top-300.

Sharding: pure data parallelism, batch 32 -> 8 cores x 4 samples.
"""

import sys

sys.path.insert(0, "/opt/trn_rl_repo")

import numpy as np

B_PER_CORE = 4
CIN = 256
CMID = 256
COUT = 65
H, W = 60, 80
PIX = H * W  # 4800
HP, WP = H + 2, W + 2  # 62, 82
PPIX = HP * WP  # 5084
GRID = 8
HH, HW_ = H * GRID, W * GRID  # 480, 640
HEAT_N = HH * HW_  # 307200
NTILE = 10  # pixel tiles for conv (480 each)
TILE_PIX = PIX // NTILE  # 480
TROWS = TILE_PIX // W  # 6 rows per tile
EPS = 1e-5

NMS_SIZE = 4.0
IOU_TH = 0.1
MIN_PROB = 0.015
TOP_K = 300
NMS_CAND = 1024

_CACHED = {}


def _last_in_maps_get():
    return _CACHED.get("last_in_maps")


def _build_nc():
    import concourse.bacc as bacc
    import concourse.mybir as mybir
    from concourse.tile import TileContext

    f32 = mybir.dt.float32
    bf16 = mybir.dt.bfloat16
    nc = bacc.Bacc("TRN2", target_bir_lowering=False, debug=False, num_devices=8)

    xh_ext = nc.declare_dram_parameter(
        "x_hi", [B_PER_CORE, CIN, H, W], bf16, isOutput=False
    )
    xl_ext = nc.declare_dram_parameter(
        "x_lo", [B_PER_CORE, CIN, H, W], bf16, isOutput=False
    )
    wa_ext = nc.declare_dram_parameter("Wa", [CMID, CIN * 9], f32, isOutput=False)
    wb_ext = nc.declare_dram_parameter("Wb", [COUT, CMID], f32, isOutput=False)
    sa_ext = nc.declare_dram_parameter("scale_a", [CMID], f32, isOutput=False)
    ba_ext = nc.declare_dram_parameter("bias_a", [CMID], f32, isOutput=False)
    sb_ext = nc.declare_dram_parameter("scale_b", [COUT, 1], f32, isOutput=False)
    bb_ext = nc.declare_dram_parameter("bias_b", [COUT, 1], f32, isOutput=False)
    ones_ext = nc.declare_dram_parameter("ones128", [128, 128], f32, isOutput=False)
    ident_ext = nc.declare_dram_parameter("ident128", [128, 128], f32, isOutput=False)

    s_dram = nc.dram_tensor("s_scratch", [B_PER_CORE, PIX], f32)
    logits_out = nc.declare_dram_parameter(
        "logits", [B_PER_CORE, COUT, H, W], f32, isOutput=True
    )
    heat_out = nc.declare_dram_parameter(
        "heat", [B_PER_CORE, HH, HW_], f32, isOutput=True
    )

    with TileContext(nc) as tc:
        with (
            tc.tile_pool(name="const", bufs=1) as cpool,
            tc.tile_pool(name="wts", bufs=1) as wpool,
            tc.tile_pool(name="xp", bufs=1) as xpool,
            tc.tile_pool(name="hb", bufs=1) as hpool,
            tc.tile_pool(name="cm", bufs=1) as cmpool,
            tc.tile_pool(name="wk", bufs=2) as wkpool,
            tc.tile_pool(name="ps", bufs=5, space="PSUM") as pspool,
            tc.tile_pool(name="ps1", bufs=2, space="PSUM") as ps1pool,
        ):
            ones_sb = cpool.tile([128, 128], f32)
            ident_sb = cpool.tile([128, 128], f32)
            nc.sync.dma_start(out=ones_sb[:], in_=ones_ext[:])
            nc.sync.dma_start(out=ident_sb[:], in_=ident_ext[:])

            # ---- weight prep ----
            # Wa laid as [co(2 chunks of 128 partitions), ci*9]; scale folded in.
            wa_sb = wpool.tile([128, 2, CIN * 9], f32)
            nc.sync.dma_start(
                out=wa_sb[:],
                in_=wa_ext.ap().rearrange("(m p) k -> p m k", p=128),
            )
            sa_sb = wpool.tile([128, 2], f32)
            nc.sync.dma_start(
                out=sa_sb[:], in_=sa_ext.ap().rearrange("(m p) -> p m", p=128)
            )
            ba_sb = wpool.tile([128, 2], f32)
            nc.sync.dma_start(
                out=ba_sb[:], in_=ba_ext.ap().rearrange("(m p) -> p m", p=128)
            )
            for m in range(2):
                nc.vector.tensor_scalar_mul(
                    wa_sb[:, m, :], wa_sb[:, m, :], sa_sb[:, m : m + 1]
                )
            # transpose -> waT split into bf16 hi/lo pair (3-pass fp32 matmul)
            # one tile per (k,t) tap so conv-a can start before prep finishes
            waT_hi = [wpool.tile([128, CMID], bf16, tag=f"wah{i}", name=f"wah{i}") for i in range(18)]
            waT_lo = [wpool.tile([128, CMID], bf16, tag=f"wal{i}", name=f"wal{i}") for i in range(18)]
            for k in range(2):
                for t in range(9):
                    for m in range(2):
                        pst = pspool.tile([128, 128], f32, tag="mm")
                        srcap = wa_sb[:, m, :].rearrange(
                            "p (c nine) -> p c nine", nine=9
                        )
                        nc.tensor.transpose(
                            pst[:], srcap[:, k * 128 : (k + 1) * 128, t], ident_sb[:]
                        )
                        dst_hi = waT_hi[k * 9 + t][:, m * 128 : (m + 1) * 128]
                        dst_lo = waT_lo[k * 9 + t][:, m * 128 : (m + 1) * 128]
                        scr = wkpool.tile([128, 128], f32, tag="wscr")
                        nc.scalar.copy(dst_hi, pst[:])  # f32 -> bf16 round
                        nc.scalar.copy(scr[:], dst_hi)  # bf16 -> f32 exact
                        nc.vector.tensor_sub(scr[:], pst[:], scr[:])
                        nc.vector.tensor_copy(dst_lo, scr[:])  # residual -> bf16

            # Wb [65, 256] scaled then transposed -> wbT [ci 128, k 2, 65]
            wb_sb = wpool.tile([COUT, CMID], f32)
            nc.sync.dma_start(out=wb_sb[:], in_=wb_ext[:])
            sb_sb = wpool.tile([COUT, 1], f32)
            nc.sync.dma_start(out=sb_sb[:], in_=sb_ext[:])
            bb_sb = wpool.tile([COUT, 1], f32)
            nc.sync.dma_start(out=bb_sb[:], in_=bb_ext[:])
            nc.vector.tensor_scalar_mul(wb_sb[:], wb_sb[:], sb_sb[:])
            wbT_hi = wpool.tile([128, 2, COUT], bf16)
            wbT_lo = wpool.tile([128, 2, COUT], bf16)
            for k in range(2):
                pst = pspool.tile([128, COUT], f32, tag="mm")
                nc.tensor.transpose(
                    pst[:], wb_sb[:, k * 128 : (k + 1) * 128], ident_sb[:COUT, :COUT]
                )
                scrb = wkpool.tile([128, COUT], f32, tag="wscr2")
                nc.scalar.copy(wbT_hi[:, k, :], pst[:])
                nc.scalar.copy(scrb[:], wbT_hi[:, k, :])
                nc.vector.tensor_sub(scrb[:], pst[:], scrb[:])
                nc.vector.tensor_copy(wbT_lo[:, k, :], scrb[:])

            # padded input tiles (border zeroed once; interior rewritten per sample)
            x_ph = xpool.tile([128, 2, PPIX], bf16)
            x_pl = xpool.tile([128, 2, PPIX], bf16)
            nc.vector.memset(x_ph[:], 0.0)
            nc.vector.memset(x_pl[:], 0.0)

            h_hi = hpool.tile([128, 2, PIX], bf16)
            h_lo = hpool.tile([128, 2, PIX], bf16)
            logits_cm = cmpool.tile([COUT, PIX], f32)
            s_row = cmpool.tile([1, PIX], f32)

            for b in range(B_PER_CORE):
                work_cm = wkpool.tile([COUT, PIX], f32, tag="work")
                # load x hi/lo into padded interiors (one DMA per ci chunk)
                for xt, xe in ((x_ph, xh_ext), (x_pl, xl_ext)):
                    for k in range(2):
                        nc.sync.dma_start(
                            out=xt[:, k, :].rearrange("p (hh ww) -> p hh ww", hh=HP)[
                                :, 1 : 1 + H, 1 : 1 + W
                            ],
                            in_=xe.ap()[b].rearrange(
                                "(k p) hh ww -> k p hh ww", p=128
                            )[k],
                        )
                # ---- conv-a (3x3) + BN + ReLU ----
                for m in range(2):
                    for tl in range(NTILE):
                        pa = pspool.tile([128, TILE_PIX], f32, tag="mm")
                        y0 = tl * TROWS
                        n_mm = 0
                        for k in range(2):
                            for t in range(9):
                                dy, dx = t // 3, t % 3

                                def _rhs(xt):
                                    return xt[:, k, :].rearrange(
                                        "p (hh ww) -> p hh ww", hh=HP
                                    )[:, y0 + dy : y0 + dy + TROWS, dx : dx + W]

                                wslice_hi = waT_hi[k * 9 + t][
                                    :, m * 128 : (m + 1) * 128
                                ]
                                wslice_lo = waT_lo[k * 9 + t][
                                    :, m * 128 : (m + 1) * 128
                                ]
                                for lhs, rhs in (
                                    (wslice_hi, _rhs(x_ph)),
                                    (wslice_hi, _rhs(x_pl)),
                                    (wslice_lo, _rhs(x_ph)),
                                ):
                                    nc.tensor.matmul(
                                        pa[:],
                                        lhs,
                                        rhs,
                                        start=(n_mm == 0),
                                        stop=(n_mm == 53),
                                    )
                                    n_mm += 1
                        hs = wkpool.tile([128, TILE_PIX], f32, tag="hscr")
                        hs2 = wkpool.tile([128, TILE_PIX], f32, tag="hscr2")
                        sl = slice(tl * TILE_PIX, (tl + 1) * TILE_PIX)
                        nc.scalar.activation(
                            hs[:],
                            pa[:],
                            mybir.ActivationFunctionType.Relu,
                            bias=ba_sb[:, m : m + 1],
                            scale=1.0,
                        )
                        nc.scalar.copy(h_hi[:, m, sl], hs[:])
                        nc.vector.tensor_copy(hs2[:], h_hi[:, m, sl])
                        nc.vector.tensor_sub(hs2[:], hs[:], hs2[:])
                        nc.vector.tensor_copy(h_lo[:, m, sl], hs2[:])

                # ---- conv-b (1x1) + BN  (channel-major) ----
                for tl in range(NTILE):
                    pb = pspool.tile([COUT, TILE_PIX], f32, tag="mm")
                    n_mm = 0
                    for k in range(2):
                        hsl = slice(tl * TILE_PIX, (tl + 1) * TILE_PIX)
                        for lhs, rhs in (
                            (wbT_hi[:, k, :], h_hi[:, k, hsl]),
                            (wbT_hi[:, k, :], h_lo[:, k, hsl]),
                            (wbT_lo[:, k, :], h_hi[:, k, hsl]),
                        ):
                            nc.tensor.matmul(
                                pb[:], lhs, rhs, start=(n_mm == 0), stop=(n_mm == 5)
                            )
                            n_mm += 1
                    nc.vector.tensor_scalar_add(
                        logits_cm[:, tl * TILE_PIX : (tl + 1) * TILE_PIX],
                        pb[:],
                        bb_sb[:],
                    )
                nc.sync.dma_start(
                    out=logits_out.ap()[b].rearrange("c hh ww -> c (hh ww)"),
                    in_=logits_cm[:],
                )

                # ---- softmax over channel dim ----
                nc.scalar.activation(
                    work_cm[:], logits_cm[:], mybir.ActivationFunctionType.Exp
                )
                for tl in range(NTILE):
                    psum_s = ps1pool.tile([1, TILE_PIX], f32, tag="sm")
                    nc.tensor.matmul(
                        psum_s[:],
                        ones_sb[:COUT, 0:1],
                        work_cm[:, tl * TILE_PIX : (tl + 1) * TILE_PIX],
                        start=True,
                        stop=True,
                    )
                    nc.vector.tensor_copy(
                        s_row[:, tl * TILE_PIX : (tl + 1) * TILE_PIX], psum_s[:]
                    )
                nc.vector.reciprocal(s_row[:], s_row[:])
                # broadcast 1/S to all channel partitions: bounce via DRAM with a
                # 0-stride read on the DRAM side; logits_cm is dead after its
                # output DMA, so reuse it as the broadcast target
                nc.sync.dma_start(out=s_dram.ap()[b][None, :], in_=s_row[:])
                nc.sync.dma_start(
                    out=logits_cm[:],
                    in_=s_dram.ap()[b][None, :].broadcast_to([COUT, PIX]),
                )
                nc.vector.tensor_mul(work_cm[:], work_cm[:], logits_cm[:])

                # ---- heat output: pixel shuffle via DMA access pattern ----
                # heat[b, hc*8+r, wc*8+cc] = prob[r*8+cc, hc*80+wc]
                dma_engines = (nc.sync, nc.scalar, nc.gpsimd)
                for r in range(GRID):
                    for cc in range(GRID):
                        c = r * GRID + cc
                        dma_engines[c % len(dma_engines)].dma_start(
                            out=heat_out.ap()[b].rearrange(
                                "(hc r) (wc cc) -> r cc hc wc", r=GRID, cc=GRID
                            )[r, cc],
                            in_=work_cm[c : c + 1, :].rearrange(
                                "c (hc wc) -> c hc wc", hc=H
                            ),
                        )

    nc.finalize()
    return nc


def _get_nc():
    if "nc" not in _CACHED:
        _CACHED["nc"] = _build_nc()
    return _CACHED["nc"]


def _host_nms(heat):
    """Exact replication of reference _box_nms (vectorized, Jacobi to fixpoint)."""
    B = heat.shape[0]
    flat = heat.reshape(B, -1)
    # top-1024 sorted desc, ties by index asc (matches jax top_k)
    idx = np.argsort(-flat, axis=1, kind="stable")[:, :NMS_CAND]
    scores = np.take_along_axis(flat, idx, axis=1)
    ys = (idx // HW_).astype(np.float32)
    xs = (idx % HW_).astype(np.float32)
    heat_nms = np.zeros_like(flat)
    for b in range(B):
        dy = np.abs(ys[b][:, None] - ys[b][None, :])
        dx = np.abs(xs[b][:, None] - xs[b][None, :])
        inter = np.maximum(NMS_SIZE - dy, 0.0) * np.maximum(NMS_SIZE - dx, 0.0)
        iou = inter / (2.0 * NMS_SIZE * NMS_SIZE - inter)
        overlap = iou > IOU_TH
        valid = scores[b] > MIN_PROB
        np.fill_diagonal(overlap, False)
        ov_ut = np.triu(overlap, 1)
        keep = valid.copy()
        for _ in range(NMS_CAND + 1):  # Jacobi fixpoint == greedy result;
            supp = ov_ut[keep].any(axis=0)  # converges in <= chain depth iters
            newkeep = valid & ~supp
            if (newkeep == keep).all():
                break
            keep = newkeep
        rank = np.cumsum(keep)
        keep = keep & (rank <= TOP_K)
        kept = np.where(keep, scores[b], 0.0).astype(np.float32)
        heat_nms[b, idx[b]] = kept
    return heat_nms.reshape(B, HH, HW_)


def kernel(x, Wa, ba, ga, bta, ma, va, Wb, bb, gb, btb, mb, vb):
    from concourse.bass_utils import run_bass_kernel_spmd

    import ml_dtypes

    x = np.ascontiguousarray(np.asarray(x, dtype=np.float32))
    x_hi = x.astype(ml_dtypes.bfloat16)
    x_lo = (x - x_hi.astype(np.float32)).astype(ml_dtypes.bfloat16)
    nc = _get_nc()

    # fold BN params on host (cheap per-channel math, not data-dependent)
    def bn_fold(g, v, m_, bt, bconv):
        g = np.asarray(g, np.float32)
        v = np.asarray(v, np.float32)
        m_ = np.asarray(m_, np.float32)
        bt = np.asarray(bt, np.float32)
        bconv = np.asarray(bconv, np.float32)
        scale = (g * (1.0 / np.sqrt(v + np.float32(EPS)))).astype(np.float32)
        bias = ((bconv - m_) * scale + bt).astype(np.float32)
        return scale, bias

    scale_a, bias_a = bn_fold(ga, va, ma, bta, ba)
    scale_b, bias_b = bn_fold(gb, vb, mb, btb, bb)

    WaR = np.ascontiguousarray(np.asarray(Wa, np.float32).reshape(CMID, CIN, 9)
                               .transpose(0, 1, 2).reshape(CMID, CIN * 9))
    WbR = np.ascontiguousarray(np.asarray(Wb, np.float32).reshape(COUT, CMID))

    ones128 = np.ones((128, 128), np.float32)
    ident128 = np.eye(128, dtype=np.float32)

    in_maps = []
    for c in range(8):
        in_maps.append(
            {
                "x_hi": x_hi[c * B_PER_CORE : (c + 1) * B_PER_CORE],
                "x_lo": x_lo[c * B_PER_CORE : (c + 1) * B_PER_CORE],
                "Wa": WaR,
                "Wb": WbR,
                "scale_a": scale_a,
                "bias_a": bias_a,
                "scale_b": scale_b.reshape(COUT, 1),
                "bias_b": bias_b.reshape(COUT, 1),
                "ones128": ones128,
                "ident128": ident128,
            }
        )

    _CACHED["last_in_maps"] = in_maps
    res = run_bass_kernel_spmd(nc, in_maps, core_ids=list(range(8)))
    logits = np.concatenate([r["logits"] for r in res.results], axis=0)
    heat = np.concatenate([r["heat"] for r in res.results], axis=0)

    heat_nms = _host_nms(heat)
    pred = (heat_nms >= MIN_PROB).astype(np.int32)
    return logits, heat, heat_nms, pred
